# revision 7
# baseline (speedup 1.0000x reference)
"""Bag-of-words histogram kernel for Trainium2 (8 NeuronCores, data-parallel).

Problem: inputs [1024, 512] int32 token ids in [0, 50257); output [1024, 50256]
f32 per-row token-count histogram with token id 0 dropped.

Design (per core, 128 rows):
  Token t decomposes as t = hi*394 + lo with hi in [0,128), lo in [0,394)
  (exact integer div via multiply-shift: hi = (t*21291)>>23, verified offline
  for all t < 50257). Per row the histogram is
      hist[hi, lo] = sum_j onehot_hi(t_j)[hi] * onehot_lo(t_j)[lo]
  which is a matmul: lhsT = A [K=128 tokens, M=128 hi-bins] f16 one-hot,
  rhs = B [K=128 tokens, N=394 lo-bins] f16 one-hot, accumulated over 4
  K-chunks of 128 tokens into one PSUM bank [128, 394] f32. Duplicate tokens
  are handled exactly by the accumulation. The PSUM tile is the output row in
  partition-major order (v = hi*394 + lo), DMAed straight to HBM in 3 pieces
  (dropping v=0 and the v>50256 tail).

  One-hots are built on DVE with tensor_scalar(is_equal) against constant
  iota rows, using per-partition f32 scalars = the transposed hi/lo digit
  of each token (DMA-transposed u16 digit tensors).
"""

import sys

sys.path.insert(0, "/opt/trn_rl_repo")

import numpy as np

N_CORES = 8
B_FULL = 1024
P = 128  # rows per core / partitions
S = 512  # tokens per row
V = 50257
R1 = 128  # hi bins (partition dim of PSUM)
R2 = 394  # lo bins (free dim of PSUM)
DIV_MUL = 21291  # (t*DIV_MUL)>>DIV_SH == t//394 for all t in [0, 50257)
DIV_SH = 23
NCH = S // P  # 4 K-chunks per row
OUT_COLS = V - 1  # 50256

# output row pieces (v = hi*R2 + lo, dropping v=0, stopping at v=50256)
PIECE1 = R2 - 1  # psum[0, 1:R2]            -> v = 1..393
PIECE2 = 126 * R2  # psum[1:127, :]          -> v = 394..50037
PIECE3 = OUT_COLS - PIECE1 - PIECE2  # psum[127, 0:PIECE3] -> v = 50038..50256

_CACHED = {}


def _build_program(reps: int = 1):
    import concourse.tile as tile
    from concourse import bacc, mybir

    nc = bacc.Bacc(
        "TRN2",
        target_bir_lowering=False,
        debug=False,
        enable_asserts=False,
        num_devices=N_CORES,
    )
    tok_dram = nc.dram_tensor("inputs", [P, S], mybir.dt.int32, kind="ExternalInput").ap()
    out_dram = nc.dram_tensor(
        "out", [P, OUT_COLS], mybir.dt.float32, kind="ExternalOutput"
    ).ap()

    alu = mybir.AluOpType

    with tile.TileContext(nc) as tc:
        with (
            tc.tile_pool(name="const", bufs=1) as const,
            tc.tile_pool(name="prep", bufs=1) as prep,
            tc.tile_pool(name="oh_a", bufs=12) as oh_a,
            tc.tile_pool(name="oh_b", bufs=12) as oh_b,
            tc.tile_pool(name="evac", bufs=6) as evac,
            tc.tile_pool(name="psum", bufs=8, space="PSUM") as psum,
        ):
            # constant iota rows (same in every partition)
            iota_i16 = const.tile([P, R2], mybir.dt.int16)
            nc.gpsimd.iota(iota_i16[:], pattern=[[1, R2]], base=0, channel_multiplier=0)
            iota_lo = const.tile([P, R2], mybir.dt.float16)
            nc.vector.tensor_copy(iota_lo[:], iota_i16[:])
            iota_hi = const.tile([P, R1], mybir.dt.float16)
            nc.vector.tensor_copy(iota_hi[:], iota_i16[:, :R1])

            # load tokens and split digits
            tok = prep.tile([P, S], mybir.dt.int32)
            nc.sync.dma_start(tok[:], tok_dram[:])
            hi32 = prep.tile([P, S], mybir.dt.int32)
            hprod = prep.tile([P, S], mybir.dt.int32)
            nc.vector.tensor_scalar(hprod[:], tok[:], DIV_MUL, None, op0=alu.mult)
            nc.vector.tensor_scalar(
                hi32[:], hprod[:], DIV_SH, None, op0=alu.logical_shift_right
            )
            lo32 = prep.tile([P, S], mybir.dt.int32)
            him = prep.tile([P, S], mybir.dt.int32)
            nc.vector.tensor_scalar(him[:], hi32[:], R2, None, op0=alu.mult)
            nc.vector.tensor_tensor(lo32[:], tok[:], him[:], op=alu.subtract)

            hi16 = prep.tile([P, S], mybir.dt.uint16)
            nc.vector.tensor_copy(hi16[:], hi32[:])
            lo16 = prep.tile([P, S], mybir.dt.uint16)
            nc.vector.tensor_copy(lo16[:], lo32[:])

            # transpose each 128-col chunk: digT[p, c*128+b] = dig[b, c*128+p]
            hiT16 = prep.tile([P, S], mybir.dt.uint16)
            loT16 = prep.tile([P, S], mybir.dt.uint16)
            for c in range(NCH):
                sl = slice(c * P, (c + 1) * P)
                nc.sync.dma_start_transpose(hiT16[:, sl], hi16[:, sl])
                nc.sync.dma_start_transpose(loT16[:, sl], lo16[:, sl])

            # f32 per-partition scalar sources
            hiT = prep.tile([P, S], mybir.dt.float32)
            nc.vector.tensor_copy(hiT[:], hiT16[:])
            loT = prep.tile([P, S], mybir.dt.float32)
            nc.vector.tensor_copy(loT[:], loT16[:])

            for b in [b for _ in range(reps) for b in range(P)]:
                ps = psum.tile([P, R2], mybir.dt.float32)
                for c in range(NCH):
                    col = c * P + b
                    a_t = oh_a.tile([P, R1], mybir.dt.float16)
                    b_t = oh_b.tile([P, R2], mybir.dt.float16)
                    nc.vector.tensor_scalar(
                        a_t[:], iota_hi[:], hiT[:, col : col + 1], None, op0=alu.is_equal
                    )
                    nc.vector.tensor_scalar(
                        b_t[:], iota_lo[:], loT[:, col : col + 1], None, op0=alu.is_equal
                    )
                    nc.tensor.matmul(
                        ps[:], a_t[:], b_t[:], start=(c == 0), stop=(c == NCH - 1)
                    )
                # DMA cannot read PSUM; evacuate via otherwise-idle ScalarE
                ev = evac.tile([P, R2], mybir.dt.float32)
                nc.scalar.copy(ev[:], ps[:])
                nc.sync.dma_start(out_dram[b : b + 1, 0:PIECE1], ev[0:1, 1:R2])
                nc.sync.dma_start(
                    out_dram[b, PIECE1 : PIECE1 + PIECE2], ev[1:127, :]
                )
                nc.sync.dma_start(
                    out_dram[b : b + 1, PIECE1 + PIECE2 : OUT_COLS],
                    ev[127:128, 0:PIECE3],
                )

    nc.compile()
    return nc


def _get_program():
    if "nc" not in _CACHED:
        _CACHED["nc"] = _build_program()
    return _CACHED["nc"]


def kernel(inputs: np.ndarray, _trace: bool = False, _tmpdir: str | None = None):
    from concourse.bass_utils import run_bass_kernel_spmd

    nc = _get_program()
    inputs = np.ascontiguousarray(np.asarray(inputs, dtype=np.int32))
    assert inputs.shape == (B_FULL, S), inputs.shape
    in_maps = [
        {"inputs": inputs[k * P : (k + 1) * P]} for k in range(N_CORES)
    ]
    res = run_bass_kernel_spmd(
        nc,
        in_maps,
        core_ids=list(range(N_CORES)),
        trace=_trace,
        tmpdir=_tmpdir,
    )
    out = np.concatenate([r["out"] for r in res.results], axis=0)
    if _trace:
        _CACHED["last_results"] = res
    return out


# revision 10
# speedup vs baseline: 173.8815x; 173.8815x over previous
"""Bag-of-words histogram kernel for Trainium2 (8 NeuronCores, data-parallel).

Problem: inputs [1024, 512] int32 token ids in [0, 50257); output [1024, 50256]
f32 per-row token-count histogram with token id 0 dropped.

Design (per core, 128 rows):
  Token t decomposes as t = hi*394 + lo with hi in [0,128), lo in [0,394)
  (exact integer div via multiply-shift: hi = (t*21291)>>23, verified offline
  for all t < 50257). Per row the histogram is
      hist[hi, lo] = sum_j onehot_hi(t_j)[hi] * onehot_lo(t_j)[lo]
  which is a matmul: lhsT = A [K=128 tokens, M=128 hi-bins] f16 one-hot,
  rhs = B [K=128 tokens, N=394 lo-bins] f16 one-hot, accumulated over 4
  K-chunks of 128 tokens into one PSUM bank [128, 394] f32. Duplicate tokens
  are handled exactly by the accumulation. The PSUM tile is the output row in
  partition-major order (v = hi*394 + lo), DMAed straight to HBM in 3 pieces
  (dropping v=0 and the v>50256 tail).

  One-hots are built on DVE with tensor_scalar(is_equal) against constant
  iota rows, using per-partition f32 scalars = the transposed hi/lo digit
  of each token (DMA-transposed u16 digit tensors).
"""

import sys

sys.path.insert(0, "/opt/trn_rl_repo")

import numpy as np

N_CORES = 8
B_FULL = 1024
P = 128  # rows per core / partitions
S = 512  # tokens per row
V = 50257
R1 = 128  # hi bins (partition dim of PSUM)
R2 = 394  # lo bins (free dim of PSUM)
DIV_MUL = 21291  # (t*DIV_MUL)>>DIV_SH == t//394 for all t in [0, 50257)
DIV_SH = 23
NCH = S // P  # 4 K-chunks per row
OUT_COLS = V - 1  # 50256

# output row pieces (v = hi*R2 + lo, dropping v=0, stopping at v=50256)
PIECE1 = R2 - 1  # psum[0, 1:R2]            -> v = 1..393
PIECE2 = 126 * R2  # psum[1:127, :]          -> v = 394..50037
PIECE3 = OUT_COLS - PIECE1 - PIECE2  # psum[127, 0:PIECE3] -> v = 50038..50256

_CACHED = {}


def _build_program(reps: int = 1):
    import concourse.tile as tile
    from concourse import bacc, mybir

    nc = bacc.Bacc(
        "TRN2",
        target_bir_lowering=False,
        debug=False,
        enable_asserts=False,
        num_devices=N_CORES,
    )
    tok_dram = nc.dram_tensor("inputs", [P, S], mybir.dt.int32, kind="ExternalInput").ap()
    out_dram = nc.dram_tensor(
        "out", [P, OUT_COLS], mybir.dt.float32, kind="ExternalOutput"
    ).ap()

    alu = mybir.AluOpType

    with tile.TileContext(nc) as tc:
        with (
            tc.tile_pool(name="const", bufs=1) as const,
            tc.tile_pool(name="prep", bufs=1) as prep,
            tc.tile_pool(name="oh_a", bufs=12) as oh_a,
            tc.tile_pool(name="oh_b", bufs=12) as oh_b,
            tc.tile_pool(name="evac", bufs=6) as evac,
            tc.tile_pool(name="psum", bufs=8, space="PSUM") as psum,
        ):
            # constant iota rows (same in every partition)
            iota_i16 = const.tile([P, R2], mybir.dt.int16)
            nc.gpsimd.iota(iota_i16[:], pattern=[[1, R2]], base=0, channel_multiplier=0)
            iota_lo = const.tile([P, R2], mybir.dt.float16)
            nc.vector.tensor_copy(iota_lo[:], iota_i16[:])
            iota_hi = const.tile([P, R1], mybir.dt.float16)
            nc.vector.tensor_copy(iota_hi[:], iota_i16[:, :R1])

            # load tokens and split digits
            tok = prep.tile([P, S], mybir.dt.int32)
            nc.sync.dma_start(tok[:], tok_dram[:])
            hi32 = prep.tile([P, S], mybir.dt.int32)
            hprod = prep.tile([P, S], mybir.dt.int32)
            nc.vector.tensor_scalar(hprod[:], tok[:], DIV_MUL, None, op0=alu.mult)
            nc.vector.tensor_scalar(
                hi32[:], hprod[:], DIV_SH, None, op0=alu.logical_shift_right
            )
            lo32 = prep.tile([P, S], mybir.dt.int32)
            him = prep.tile([P, S], mybir.dt.int32)
            nc.vector.tensor_scalar(him[:], hi32[:], R2, None, op0=alu.mult)
            nc.vector.tensor_tensor(lo32[:], tok[:], him[:], op=alu.subtract)

            hi16 = prep.tile([P, S], mybir.dt.uint16)
            nc.vector.tensor_copy(hi16[:], hi32[:])
            lo16 = prep.tile([P, S], mybir.dt.uint16)
            nc.vector.tensor_copy(lo16[:], lo32[:])

            # transpose each 128-col chunk: digT[p, c*128+b] = dig[b, c*128+p]
            hiT16 = prep.tile([P, S], mybir.dt.uint16)
            loT16 = prep.tile([P, S], mybir.dt.uint16)
            for c in range(NCH):
                sl = slice(c * P, (c + 1) * P)
                nc.sync.dma_start_transpose(hiT16[:, sl], hi16[:, sl])
                nc.sync.dma_start_transpose(loT16[:, sl], lo16[:, sl])

            # f32 per-partition scalar sources
            hiT = prep.tile([P, S], mybir.dt.float32)
            nc.vector.tensor_copy(hiT[:], hiT16[:])
            loT = prep.tile([P, S], mybir.dt.float32)
            nc.vector.tensor_copy(loT[:], loT16[:])

            for b in [b for _ in range(reps) for b in range(P)]:
                ps = psum.tile([P, R2], mybir.dt.float32)
                for c in range(NCH):
                    col = c * P + b
                    a_t = oh_a.tile([P, R1], mybir.dt.float16)
                    b_t = oh_b.tile([P, R2], mybir.dt.float16)
                    nc.vector.tensor_scalar(
                        a_t[:], iota_hi[:], hiT[:, col : col + 1], None, op0=alu.is_equal
                    )
                    nc.vector.tensor_scalar(
                        b_t[:], iota_lo[:], loT[:, col : col + 1], None, op0=alu.is_equal
                    )
                    nc.tensor.matmul(
                        ps[:], a_t[:], b_t[:], start=(c == 0), stop=(c == NCH - 1)
                    )
                # DMA cannot read PSUM; evacuate via otherwise-idle ScalarE
                ev = evac.tile([P, R2], mybir.dt.float32)
                nc.scalar.copy(ev[:], ps[:])
                nc.sync.dma_start(out_dram[b : b + 1, 0:PIECE1], ev[0:1, 1:R2])
                nc.sync.dma_start(
                    out_dram[b, PIECE1 : PIECE1 + PIECE2], ev[1:127, :]
                )
                nc.sync.dma_start(
                    out_dram[b : b + 1, PIECE1 + PIECE2 : OUT_COLS],
                    ev[127:128, 0:PIECE3],
                )

    nc.compile()
    return nc


def _build_program_dyn():
    """Variant with a runtime repeat loop around the row loop, for HW timing.

    Trip count comes from the extra [1,1] uint32 input "reps" — same NEFF for
    any R, so wall-time slope over R isolates device execution time.
    """
    import concourse.tile as tile
    from concourse import bacc, mybir

    nc = bacc.Bacc(
        "TRN2",
        target_bir_lowering=False,
        debug=False,
        enable_asserts=False,
        num_devices=N_CORES,
    )
    tok_dram = nc.dram_tensor("inputs", [P, S], mybir.dt.int32, kind="ExternalInput").ap()
    reps_dram = nc.dram_tensor("reps", [1, 1], mybir.dt.uint32, kind="ExternalInput").ap()
    out_dram = nc.dram_tensor(
        "out", [P, OUT_COLS], mybir.dt.float32, kind="ExternalOutput"
    ).ap()

    alu = mybir.AluOpType

    with tile.TileContext(nc) as tc:
        with (
            tc.tile_pool(name="const", bufs=1) as const,
            tc.tile_pool(name="prep", bufs=1) as prep,
            tc.tile_pool(name="oh_a", bufs=12) as oh_a,
            tc.tile_pool(name="oh_b", bufs=12) as oh_b,
            tc.tile_pool(name="evac", bufs=6) as evac,
            tc.tile_pool(name="psum", bufs=8, space="PSUM") as psum,
        ):
            iota_i16 = const.tile([P, R2], mybir.dt.int16)
            nc.gpsimd.iota(iota_i16[:], pattern=[[1, R2]], base=0, channel_multiplier=0)
            iota_lo = const.tile([P, R2], mybir.dt.float16)
            nc.vector.tensor_copy(iota_lo[:], iota_i16[:])
            iota_hi = const.tile([P, R1], mybir.dt.float16)
            nc.vector.tensor_copy(iota_hi[:], iota_i16[:, :R1])

            reps_sb = const.tile([1, 1], mybir.dt.uint32)
            nc.sync.dma_start(reps_sb[:], reps_dram[:])

            tok = prep.tile([P, S], mybir.dt.int32)
            nc.sync.dma_start(tok[:], tok_dram[:])
            hi32 = prep.tile([P, S], mybir.dt.int32)
            hprod = prep.tile([P, S], mybir.dt.int32)
            nc.vector.tensor_scalar(hprod[:], tok[:], DIV_MUL, None, op0=alu.mult)
            nc.vector.tensor_scalar(
                hi32[:], hprod[:], DIV_SH, None, op0=alu.logical_shift_right
            )
            lo32 = prep.tile([P, S], mybir.dt.int32)
            him = prep.tile([P, S], mybir.dt.int32)
            nc.vector.tensor_scalar(him[:], hi32[:], R2, None, op0=alu.mult)
            nc.vector.tensor_tensor(lo32[:], tok[:], him[:], op=alu.subtract)

            hi16 = prep.tile([P, S], mybir.dt.uint16)
            nc.vector.tensor_copy(hi16[:], hi32[:])
            lo16 = prep.tile([P, S], mybir.dt.uint16)
            nc.vector.tensor_copy(lo16[:], lo32[:])

            hiT16 = prep.tile([P, S], mybir.dt.uint16)
            loT16 = prep.tile([P, S], mybir.dt.uint16)
            for c in range(NCH):
                sl = slice(c * P, (c + 1) * P)
                nc.sync.dma_start_transpose(hiT16[:, sl], hi16[:, sl])
                nc.sync.dma_start_transpose(loT16[:, sl], lo16[:, sl])

            hiT = prep.tile([P, S], mybir.dt.float32)
            nc.vector.tensor_copy(hiT[:], hiT16[:])
            loT = prep.tile([P, S], mybir.dt.float32)
            nc.vector.tensor_copy(loT[:], loT16[:])

            # load the loop bound into a register on every engine so the
            # all-engine For_i accepts it
            from concourse.bass_primitives_rust import RegisterHandles
            from concourse.expressions import make_scalar_value

            regs = []
            for eng in (nc.sync, nc.vector, nc.scalar, nc.tensor, nc.gpsimd):
                tmp = eng.alloc_register(f"reps_{eng.engine.value}")
                eng.reg_load(tmp, reps_sb[0:1, 0:1])
                regs.append(tmp)
            rv = make_scalar_value(
                RegisterHandles(regs), min_val=0, max_val=1 << 20
            )
            with tc.For_i(0, rv, 1):
                for b in range(P):
                    ps = psum.tile([P, R2], mybir.dt.float32)
                    for c in range(NCH):
                        col = c * P + b
                        a_t = oh_a.tile([P, R1], mybir.dt.float16)
                        b_t = oh_b.tile([P, R2], mybir.dt.float16)
                        nc.vector.tensor_scalar(
                            a_t[:], iota_hi[:], hiT[:, col : col + 1], None,
                            op0=alu.is_equal,
                        )
                        nc.vector.tensor_scalar(
                            b_t[:], iota_lo[:], loT[:, col : col + 1], None,
                            op0=alu.is_equal,
                        )
                        nc.tensor.matmul(
                            ps[:], a_t[:], b_t[:], start=(c == 0), stop=(c == NCH - 1)
                        )
                    ev = evac.tile([P, R2], mybir.dt.float32)
                    nc.scalar.copy(ev[:], ps[:])
                    nc.sync.dma_start(out_dram[b : b + 1, 0:PIECE1], ev[0:1, 1:R2])
                    nc.sync.dma_start(
                        out_dram[b, PIECE1 : PIECE1 + PIECE2], ev[1:127, :]
                    )
                    nc.sync.dma_start(
                        out_dram[b : b + 1, PIECE1 + PIECE2 : OUT_COLS],
                        ev[127:128, 0:PIECE3],
                    )

    nc.compile()
    return nc


def _get_program():
    if "nc" not in _CACHED:
        _CACHED["nc"] = _build_program()
    return _CACHED["nc"]


def kernel(inputs: np.ndarray, _trace: bool = False, _tmpdir: str | None = None):
    from concourse.bass_utils import run_bass_kernel_spmd

    nc = _get_program()
    inputs = np.ascontiguousarray(np.asarray(inputs, dtype=np.int32))
    assert inputs.shape == (B_FULL, S), inputs.shape
    in_maps = [
        {"inputs": inputs[k * P : (k + 1) * P]} for k in range(N_CORES)
    ]
    res = run_bass_kernel_spmd(
        nc,
        in_maps,
        core_ids=list(range(N_CORES)),
        trace=_trace,
        tmpdir=_tmpdir,
    )
    out = np.concatenate([r["out"] for r in res.results], axis=0)
    if _trace:
        _CACHED["last_results"] = res
    return out


# revision 11
# speedup vs baseline: 425.6799x; 2.4481x over previous
"""Bag-of-words histogram kernel for Trainium2 (8 NeuronCores, data-parallel).

Problem: inputs [1024, 512] int32 token ids in [0, 50257); output [1024, 50256]
f32 per-row token-count histogram with token id 0 dropped.

Design (per core, 128 rows):
  Token t decomposes as t = hi*394 + lo with hi in [0,128), lo in [0,394)
  (exact integer div via multiply-shift: hi = (t*21291)>>23, verified offline
  for all t < 50257). Per row the histogram is
      hist[hi, lo] = sum_j onehot_hi(t_j)[hi] * onehot_lo(t_j)[lo]
  which is a matmul: lhsT = A [K=128 tokens, M=128 hi-bins] f16 one-hot,
  rhs = B [K=128 tokens, N=394 lo-bins] f16 one-hot, accumulated over 4
  K-chunks of 128 tokens into one PSUM bank [128, 394] f32. Duplicate tokens
  are handled exactly by the accumulation. The PSUM tile is the output row in
  partition-major order (v = hi*394 + lo), DMAed straight to HBM in 3 pieces
  (dropping v=0 and the v>50256 tail).

  One-hots are built on DVE with tensor_scalar(is_equal) against constant
  iota rows, using per-partition f32 scalars = the transposed hi/lo digit
  of each token (DMA-transposed u16 digit tensors).
"""

import sys

sys.path.insert(0, "/opt/trn_rl_repo")

import numpy as np

N_CORES = 8
B_FULL = 1024
P = 128  # rows per core / partitions
S = 512  # tokens per row
V = 50257
R1 = 128  # hi bins (partition dim of PSUM)
R2 = 394  # lo bins (free dim of PSUM)
DIV_MUL = 21291  # (t*DIV_MUL)>>DIV_SH == t//394 for all t in [0, 50257)
DIV_SH = 23
NCH = S // P  # 4 K-chunks per row
OUT_COLS = V - 1  # 50256

# output row pieces (v = hi*R2 + lo, dropping v=0, stopping at v=50256)
PIECE1 = R2 - 1  # psum[0, 1:R2]            -> v = 1..393
PIECE2 = 126 * R2  # psum[1:127, :]          -> v = 394..50037
PIECE3 = OUT_COLS - PIECE1 - PIECE2  # psum[127, 0:PIECE3] -> v = 50038..50256

_CACHED = {}


def _build_program(reps: int = 1):
    import concourse.tile as tile
    from concourse import bacc, mybir

    nc = bacc.Bacc(
        "TRN2",
        target_bir_lowering=False,
        debug=False,
        enable_asserts=False,
        num_devices=N_CORES,
    )
    tok_dram = nc.dram_tensor("inputs", [P, S], mybir.dt.int32, kind="ExternalInput").ap()
    out_dram = nc.dram_tensor(
        "out", [P, OUT_COLS], mybir.dt.float32, kind="ExternalOutput"
    ).ap()

    alu = mybir.AluOpType

    with tile.TileContext(nc) as tc:
        with (
            tc.tile_pool(name="const", bufs=1) as const,
            tc.tile_pool(name="prep", bufs=1) as prep,
            tc.tile_pool(name="oh_a", bufs=20) as oh_a,
            tc.tile_pool(name="oh_b", bufs=20) as oh_b,
            tc.tile_pool(name="evac", bufs=8) as evac,
            tc.tile_pool(name="psum", bufs=8, space="PSUM") as psum,
        ):
            # constant iota rows (same in every partition)
            iota_i16 = const.tile([P, R2], mybir.dt.int16)
            nc.gpsimd.iota(iota_i16[:], pattern=[[1, R2]], base=0, channel_multiplier=0)
            iota_lo = const.tile([P, R2], mybir.dt.float16)
            nc.vector.tensor_copy(iota_lo[:], iota_i16[:])
            iota_hi = const.tile([P, R1], mybir.dt.float16)
            nc.vector.tensor_copy(iota_hi[:], iota_i16[:, :R1])

            # load tokens and split digits
            tok = prep.tile([P, S], mybir.dt.int32)
            nc.sync.dma_start(tok[:], tok_dram[:])
            hi32 = prep.tile([P, S], mybir.dt.int32)
            hprod = prep.tile([P, S], mybir.dt.int32)
            nc.vector.tensor_scalar(hprod[:], tok[:], DIV_MUL, None, op0=alu.mult)
            nc.vector.tensor_scalar(
                hi32[:], hprod[:], DIV_SH, None, op0=alu.logical_shift_right
            )
            lo32 = prep.tile([P, S], mybir.dt.int32)
            him = prep.tile([P, S], mybir.dt.int32)
            nc.vector.tensor_scalar(him[:], hi32[:], R2, None, op0=alu.mult)
            nc.vector.tensor_tensor(lo32[:], tok[:], him[:], op=alu.subtract)

            hi16 = prep.tile([P, S], mybir.dt.uint16)
            nc.vector.tensor_copy(hi16[:], hi32[:])
            lo16 = prep.tile([P, S], mybir.dt.uint16)
            nc.vector.tensor_copy(lo16[:], lo32[:])

            # transpose each 128-col chunk: digT[p, c*128+b] = dig[b, c*128+p]
            hiT16 = prep.tile([P, S], mybir.dt.uint16)
            loT16 = prep.tile([P, S], mybir.dt.uint16)
            for c in range(NCH):
                sl = slice(c * P, (c + 1) * P)
                nc.sync.dma_start_transpose(hiT16[:, sl], hi16[:, sl])
                nc.sync.dma_start_transpose(loT16[:, sl], lo16[:, sl])

            # f32 per-partition scalar sources
            hiT = prep.tile([P, S], mybir.dt.float32)
            nc.vector.tensor_copy(hiT[:], hiT16[:])
            loT = prep.tile([P, S], mybir.dt.float32)
            nc.vector.tensor_copy(loT[:], loT16[:])

            for b in [b for _ in range(reps) for b in range(P)]:
                ps = psum.tile([P, R2], mybir.dt.float32)
                for c in range(NCH):
                    col = c * P + b
                    a_t = oh_a.tile([P, R1], mybir.dt.float16)
                    b_t = oh_b.tile([P, R2], mybir.dt.float16)
                    nc.vector.tensor_scalar(
                        a_t[:], iota_hi[:], hiT[:, col : col + 1], None, op0=alu.is_equal
                    )
                    nc.vector.tensor_scalar(
                        b_t[:], iota_lo[:], loT[:, col : col + 1], None, op0=alu.is_equal
                    )
                    nc.tensor.matmul(
                        ps[:], a_t[:], b_t[:], start=(c == 0), stop=(c == NCH - 1)
                    )
                # DMA cannot read PSUM; evacuate via otherwise-idle ScalarE
                ev = evac.tile([P, R2], mybir.dt.float32)
                nc.scalar.copy(ev[:], ps[:])
                nc.sync.dma_start(out_dram[b : b + 1, 0:PIECE1], ev[0:1, 1:R2])
                nc.sync.dma_start(
                    out_dram[b, PIECE1 : PIECE1 + PIECE2], ev[1:127, :]
                )
                nc.sync.dma_start(
                    out_dram[b : b + 1, PIECE1 + PIECE2 : OUT_COLS],
                    ev[127:128, 0:PIECE3],
                )

    nc.compile()
    return nc


def _build_program_dyn():
    """Variant with a runtime repeat loop around the row loop, for HW timing.

    Trip count comes from the extra [1,1] uint32 input "reps" — same NEFF for
    any R, so wall-time slope over R isolates device execution time.
    """
    import concourse.tile as tile
    from concourse import bacc, mybir

    nc = bacc.Bacc(
        "TRN2",
        target_bir_lowering=False,
        debug=False,
        enable_asserts=False,
        num_devices=N_CORES,
    )
    tok_dram = nc.dram_tensor("inputs", [P, S], mybir.dt.int32, kind="ExternalInput").ap()
    reps_dram = nc.dram_tensor("reps", [1, 1], mybir.dt.uint32, kind="ExternalInput").ap()
    out_dram = nc.dram_tensor(
        "out", [P, OUT_COLS], mybir.dt.float32, kind="ExternalOutput"
    ).ap()

    alu = mybir.AluOpType

    with tile.TileContext(nc) as tc:
        with (
            tc.tile_pool(name="const", bufs=1) as const,
            tc.tile_pool(name="prep", bufs=1) as prep,
            tc.tile_pool(name="oh_a", bufs=20) as oh_a,
            tc.tile_pool(name="oh_b", bufs=20) as oh_b,
            tc.tile_pool(name="evac", bufs=8) as evac,
            tc.tile_pool(name="psum", bufs=8, space="PSUM") as psum,
        ):
            iota_i16 = const.tile([P, R2], mybir.dt.int16)
            nc.gpsimd.iota(iota_i16[:], pattern=[[1, R2]], base=0, channel_multiplier=0)
            iota_lo = const.tile([P, R2], mybir.dt.float16)
            nc.vector.tensor_copy(iota_lo[:], iota_i16[:])
            iota_hi = const.tile([P, R1], mybir.dt.float16)
            nc.vector.tensor_copy(iota_hi[:], iota_i16[:, :R1])

            reps_sb = const.tile([1, 1], mybir.dt.uint32)
            nc.sync.dma_start(reps_sb[:], reps_dram[:])

            tok = prep.tile([P, S], mybir.dt.int32)
            nc.sync.dma_start(tok[:], tok_dram[:])
            hi32 = prep.tile([P, S], mybir.dt.int32)
            hprod = prep.tile([P, S], mybir.dt.int32)
            nc.vector.tensor_scalar(hprod[:], tok[:], DIV_MUL, None, op0=alu.mult)
            nc.vector.tensor_scalar(
                hi32[:], hprod[:], DIV_SH, None, op0=alu.logical_shift_right
            )
            lo32 = prep.tile([P, S], mybir.dt.int32)
            him = prep.tile([P, S], mybir.dt.int32)
            nc.vector.tensor_scalar(him[:], hi32[:], R2, None, op0=alu.mult)
            nc.vector.tensor_tensor(lo32[:], tok[:], him[:], op=alu.subtract)

            hi16 = prep.tile([P, S], mybir.dt.uint16)
            nc.vector.tensor_copy(hi16[:], hi32[:])
            lo16 = prep.tile([P, S], mybir.dt.uint16)
            nc.vector.tensor_copy(lo16[:], lo32[:])

            hiT16 = prep.tile([P, S], mybir.dt.uint16)
            loT16 = prep.tile([P, S], mybir.dt.uint16)
            for c in range(NCH):
                sl = slice(c * P, (c + 1) * P)
                nc.sync.dma_start_transpose(hiT16[:, sl], hi16[:, sl])
                nc.sync.dma_start_transpose(loT16[:, sl], lo16[:, sl])

            hiT = prep.tile([P, S], mybir.dt.float32)
            nc.vector.tensor_copy(hiT[:], hiT16[:])
            loT = prep.tile([P, S], mybir.dt.float32)
            nc.vector.tensor_copy(loT[:], loT16[:])

            # load the loop bound into a register on every engine so the
            # all-engine For_i accepts it
            from concourse.bass_primitives_rust import RegisterHandles
            from concourse.expressions import make_scalar_value

            regs = []
            for eng in (nc.sync, nc.vector, nc.scalar, nc.tensor, nc.gpsimd):
                tmp = eng.alloc_register(f"reps_{eng.engine.value}")
                eng.reg_load(tmp, reps_sb[0:1, 0:1])
                regs.append(tmp)
            rv = make_scalar_value(
                RegisterHandles(regs), min_val=0, max_val=1 << 20
            )
            with tc.For_i(0, rv, 1):
                for b in range(P):
                    ps = psum.tile([P, R2], mybir.dt.float32)
                    for c in range(NCH):
                        col = c * P + b
                        a_t = oh_a.tile([P, R1], mybir.dt.float16)
                        b_t = oh_b.tile([P, R2], mybir.dt.float16)
                        nc.vector.tensor_scalar(
                            a_t[:], iota_hi[:], hiT[:, col : col + 1], None,
                            op0=alu.is_equal,
                        )
                        nc.vector.tensor_scalar(
                            b_t[:], iota_lo[:], loT[:, col : col + 1], None,
                            op0=alu.is_equal,
                        )
                        nc.tensor.matmul(
                            ps[:], a_t[:], b_t[:], start=(c == 0), stop=(c == NCH - 1)
                        )
                    ev = evac.tile([P, R2], mybir.dt.float32)
                    nc.scalar.copy(ev[:], ps[:])
                    nc.sync.dma_start(out_dram[b : b + 1, 0:PIECE1], ev[0:1, 1:R2])
                    nc.sync.dma_start(
                        out_dram[b, PIECE1 : PIECE1 + PIECE2], ev[1:127, :]
                    )
                    nc.sync.dma_start(
                        out_dram[b : b + 1, PIECE1 + PIECE2 : OUT_COLS],
                        ev[127:128, 0:PIECE3],
                    )

    nc.compile()
    return nc


def _get_program():
    if "nc" not in _CACHED:
        _CACHED["nc"] = _build_program()
    return _CACHED["nc"]


def kernel(inputs: np.ndarray, _trace: bool = False, _tmpdir: str | None = None):
    from concourse.bass_utils import run_bass_kernel_spmd

    nc = _get_program()
    inputs = np.ascontiguousarray(np.asarray(inputs, dtype=np.int32))
    assert inputs.shape == (B_FULL, S), inputs.shape
    in_maps = [
        {"inputs": inputs[k * P : (k + 1) * P]} for k in range(N_CORES)
    ]
    res = run_bass_kernel_spmd(
        nc,
        in_maps,
        core_ids=list(range(N_CORES)),
        trace=_trace,
        tmpdir=_tmpdir,
    )
    out = np.concatenate([r["out"] for r in res.results], axis=0)
    if _trace:
        _CACHED["last_results"] = res
    return out


# revision 13
# speedup vs baseline: 511.7983x; 1.2023x over previous
"""Bag-of-words histogram kernel for Trainium2 (8 NeuronCores, data-parallel).

Problem: inputs [1024, 512] int32 token ids in [0, 50257); output [1024, 50256]
f32 per-row token-count histogram with token id 0 dropped.

Design (per core, 128 rows):
  Token t decomposes as t = hi*394 + lo with hi in [0,128), lo in [0,394)
  (exact integer div via multiply-shift: hi = (t*21291)>>23, verified offline
  for all t < 50257). Per row the histogram is
      hist[hi, lo] = sum_j onehot_hi(t_j)[hi] * onehot_lo(t_j)[lo]
  which is a matmul: lhsT = A [K=128 tokens, M=128 hi-bins] f16 one-hot,
  rhs = B [K=128 tokens, N=394 lo-bins] f16 one-hot, accumulated over 4
  K-chunks of 128 tokens into one PSUM bank [128, 394] f32. Duplicate tokens
  are handled exactly by the accumulation. The PSUM tile is the output row in
  partition-major order (v = hi*394 + lo), DMAed straight to HBM in 3 pieces
  (dropping v=0 and the v>50256 tail).

  One-hots are built on DVE with tensor_scalar(is_equal) against constant
  iota rows, using per-partition f32 scalars = the transposed hi/lo digit
  of each token (DMA-transposed u16 digit tensors).
"""

import sys

sys.path.insert(0, "/opt/trn_rl_repo")

import numpy as np

N_CORES = 8
B_FULL = 1024
P = 128  # rows per core / partitions
S = 512  # tokens per row
V = 50257
R1 = 128  # hi bins (partition dim of PSUM)
R2 = 394  # lo bins (free dim of PSUM)
DIV_MUL = 21291  # (t*DIV_MUL)>>DIV_SH == t//394 for all t in [0, 50257)
DIV_SH = 23
NCH = S // P  # 4 K-chunks per row
OUT_COLS = V - 1  # 50256

# output row pieces (v = hi*R2 + lo, dropping v=0, stopping at v=50256)
PIECE1 = R2 - 1  # psum[0, 1:R2]            -> v = 1..393
PIECE2 = 126 * R2  # psum[1:127, :]          -> v = 394..50037
PIECE3 = OUT_COLS - PIECE1 - PIECE2  # psum[127, 0:PIECE3] -> v = 50038..50256

_CACHED = {}


def _build_program(reps: int = 1):
    import concourse.tile as tile
    from concourse import bacc, mybir

    nc = bacc.Bacc(
        "TRN2",
        target_bir_lowering=False,
        debug=False,
        enable_asserts=False,
        num_devices=N_CORES,
    )
    tok_dram = nc.dram_tensor("inputs", [P, S], mybir.dt.int32, kind="ExternalInput").ap()
    out_dram = nc.dram_tensor(
        "out", [P, OUT_COLS], mybir.dt.float32, kind="ExternalOutput"
    ).ap()

    alu = mybir.AluOpType

    with tile.TileContext(nc) as tc:
        with (
            tc.tile_pool(name="const", bufs=1) as const,
            tc.tile_pool(name="prep", bufs=1) as prep,
            tc.tile_pool(name="oh_a", bufs=20) as oh_a,
            tc.tile_pool(name="oh_b", bufs=20) as oh_b,
            tc.tile_pool(name="evac", bufs=8) as evac,
            tc.tile_pool(name="psum", bufs=8, space="PSUM") as psum,
        ):
            # constant iota rows (same in every partition)
            iota_i16 = const.tile([P, R2], mybir.dt.int16)
            nc.gpsimd.iota(iota_i16[:], pattern=[[1, R2]], base=0, channel_multiplier=0)
            iota_lo = const.tile([P, R2], mybir.dt.float16)
            nc.vector.tensor_copy(iota_lo[:], iota_i16[:])
            iota_hi = const.tile([P, R1], mybir.dt.float16)
            nc.vector.tensor_copy(iota_hi[:], iota_i16[:, :R1])

            # load tokens and split digits
            tok = prep.tile([P, S], mybir.dt.int32)
            nc.sync.dma_start(tok[:], tok_dram[:])
            hi32 = prep.tile([P, S], mybir.dt.int32)
            hprod = prep.tile([P, S], mybir.dt.int32)
            nc.vector.tensor_scalar(hprod[:], tok[:], DIV_MUL, None, op0=alu.mult)
            nc.vector.tensor_scalar(
                hi32[:], hprod[:], DIV_SH, None, op0=alu.logical_shift_right
            )
            lo32 = prep.tile([P, S], mybir.dt.int32)
            him = prep.tile([P, S], mybir.dt.int32)
            nc.vector.tensor_scalar(him[:], hi32[:], R2, None, op0=alu.mult)
            nc.vector.tensor_tensor(lo32[:], tok[:], him[:], op=alu.subtract)

            hi16 = prep.tile([P, S], mybir.dt.uint16)
            nc.vector.tensor_copy(hi16[:], hi32[:])
            lo16 = prep.tile([P, S], mybir.dt.uint16)
            nc.vector.tensor_copy(lo16[:], lo32[:])

            # transpose each 128-col chunk: digT[p, c*128+b] = dig[b, c*128+p]
            hiT16 = prep.tile([P, S], mybir.dt.uint16)
            loT16 = prep.tile([P, S], mybir.dt.uint16)
            for c in range(NCH):
                sl = slice(c * P, (c + 1) * P)
                nc.sync.dma_start_transpose(hiT16[:, sl], hi16[:, sl])
                nc.sync.dma_start_transpose(loT16[:, sl], lo16[:, sl])

            # f32 per-partition scalar sources
            hiT = prep.tile([P, S], mybir.dt.float32)
            nc.vector.tensor_copy(hiT[:], hiT16[:])
            loT = prep.tile([P, S], mybir.dt.float32)
            nc.vector.tensor_copy(loT[:], loT16[:])

            for b in [b for _ in range(reps) for b in range(P)]:
                ps = psum.tile([P, R2], mybir.dt.float32)
                for c in range(NCH):
                    col = c * P + b
                    a_t = oh_a.tile([P, R1], mybir.dt.float16)
                    b_t = oh_b.tile([P, R2], mybir.dt.float16)
                    nc.vector.tensor_scalar(
                        a_t[:], iota_hi[:], hiT[:, col : col + 1], None, op0=alu.is_equal
                    )
                    nc.vector.tensor_scalar(
                        b_t[:], iota_lo[:], loT[:, col : col + 1], None, op0=alu.is_equal
                    )
                    nc.tensor.matmul(
                        ps[:], a_t[:], b_t[:], start=(c == 0), stop=(c == NCH - 1)
                    )
                # DMA cannot read PSUM; evacuate via otherwise-idle ScalarE
                ev = evac.tile([P, R2], mybir.dt.float32)
                nc.scalar.copy(ev[:], ps[:])
                # alternate the big middle piece across both HWDGE rings
                big, small = (nc.sync, nc.scalar) if b % 2 == 0 else (nc.scalar, nc.sync)
                small.dma_start(out_dram[b : b + 1, 0:PIECE1], ev[0:1, 1:R2])
                big.dma_start(
                    out_dram[b, PIECE1 : PIECE1 + PIECE2], ev[1:127, :]
                )
                small.dma_start(
                    out_dram[b : b + 1, PIECE1 + PIECE2 : OUT_COLS],
                    ev[127:128, 0:PIECE3],
                )

    nc.compile()
    return nc


def _build_program_dyn():
    """Variant with a runtime repeat loop around the row loop, for HW timing.

    Trip count comes from the extra [1,1] uint32 input "reps" — same NEFF for
    any R, so wall-time slope over R isolates device execution time.
    """
    import concourse.tile as tile
    from concourse import bacc, mybir

    nc = bacc.Bacc(
        "TRN2",
        target_bir_lowering=False,
        debug=False,
        enable_asserts=False,
        num_devices=N_CORES,
    )
    tok_dram = nc.dram_tensor("inputs", [P, S], mybir.dt.int32, kind="ExternalInput").ap()
    reps_dram = nc.dram_tensor("reps", [1, 1], mybir.dt.uint32, kind="ExternalInput").ap()
    out_dram = nc.dram_tensor(
        "out", [P, OUT_COLS], mybir.dt.float32, kind="ExternalOutput"
    ).ap()

    alu = mybir.AluOpType

    with tile.TileContext(nc) as tc:
        with (
            tc.tile_pool(name="const", bufs=1) as const,
            tc.tile_pool(name="prep", bufs=1) as prep,
            tc.tile_pool(name="oh_a", bufs=20) as oh_a,
            tc.tile_pool(name="oh_b", bufs=20) as oh_b,
            tc.tile_pool(name="evac", bufs=8) as evac,
            tc.tile_pool(name="psum", bufs=8, space="PSUM") as psum,
        ):
            iota_i16 = const.tile([P, R2], mybir.dt.int16)
            nc.gpsimd.iota(iota_i16[:], pattern=[[1, R2]], base=0, channel_multiplier=0)
            iota_lo = const.tile([P, R2], mybir.dt.float16)
            nc.vector.tensor_copy(iota_lo[:], iota_i16[:])
            iota_hi = const.tile([P, R1], mybir.dt.float16)
            nc.vector.tensor_copy(iota_hi[:], iota_i16[:, :R1])

            reps_sb = const.tile([1, 1], mybir.dt.uint32)
            nc.sync.dma_start(reps_sb[:], reps_dram[:])

            tok = prep.tile([P, S], mybir.dt.int32)
            nc.sync.dma_start(tok[:], tok_dram[:])
            hi32 = prep.tile([P, S], mybir.dt.int32)
            hprod = prep.tile([P, S], mybir.dt.int32)
            nc.vector.tensor_scalar(hprod[:], tok[:], DIV_MUL, None, op0=alu.mult)
            nc.vector.tensor_scalar(
                hi32[:], hprod[:], DIV_SH, None, op0=alu.logical_shift_right
            )
            lo32 = prep.tile([P, S], mybir.dt.int32)
            him = prep.tile([P, S], mybir.dt.int32)
            nc.vector.tensor_scalar(him[:], hi32[:], R2, None, op0=alu.mult)
            nc.vector.tensor_tensor(lo32[:], tok[:], him[:], op=alu.subtract)

            hi16 = prep.tile([P, S], mybir.dt.uint16)
            nc.vector.tensor_copy(hi16[:], hi32[:])
            lo16 = prep.tile([P, S], mybir.dt.uint16)
            nc.vector.tensor_copy(lo16[:], lo32[:])

            hiT16 = prep.tile([P, S], mybir.dt.uint16)
            loT16 = prep.tile([P, S], mybir.dt.uint16)
            for c in range(NCH):
                sl = slice(c * P, (c + 1) * P)
                nc.sync.dma_start_transpose(hiT16[:, sl], hi16[:, sl])
                nc.sync.dma_start_transpose(loT16[:, sl], lo16[:, sl])

            hiT = prep.tile([P, S], mybir.dt.float32)
            nc.vector.tensor_copy(hiT[:], hiT16[:])
            loT = prep.tile([P, S], mybir.dt.float32)
            nc.vector.tensor_copy(loT[:], loT16[:])

            # load the loop bound into a register on every engine so the
            # all-engine For_i accepts it
            from concourse.bass_primitives_rust import RegisterHandles
            from concourse.expressions import make_scalar_value

            regs = []
            for eng in (nc.sync, nc.vector, nc.scalar, nc.tensor, nc.gpsimd):
                tmp = eng.alloc_register(f"reps_{eng.engine.value}")
                eng.reg_load(tmp, reps_sb[0:1, 0:1])
                regs.append(tmp)
            rv = make_scalar_value(
                RegisterHandles(regs), min_val=0, max_val=1 << 20
            )
            with tc.For_i(0, rv, 1):
                for b in range(P):
                    ps = psum.tile([P, R2], mybir.dt.float32)
                    for c in range(NCH):
                        col = c * P + b
                        a_t = oh_a.tile([P, R1], mybir.dt.float16)
                        b_t = oh_b.tile([P, R2], mybir.dt.float16)
                        nc.vector.tensor_scalar(
                            a_t[:], iota_hi[:], hiT[:, col : col + 1], None,
                            op0=alu.is_equal,
                        )
                        nc.vector.tensor_scalar(
                            b_t[:], iota_lo[:], loT[:, col : col + 1], None,
                            op0=alu.is_equal,
                        )
                        nc.tensor.matmul(
                            ps[:], a_t[:], b_t[:], start=(c == 0), stop=(c == NCH - 1)
                        )
                    ev = evac.tile([P, R2], mybir.dt.float32)
                    nc.scalar.copy(ev[:], ps[:])
                    big, small = (
                        (nc.sync, nc.scalar) if b % 2 == 0 else (nc.scalar, nc.sync)
                    )
                    small.dma_start(out_dram[b : b + 1, 0:PIECE1], ev[0:1, 1:R2])
                    big.dma_start(
                        out_dram[b, PIECE1 : PIECE1 + PIECE2], ev[1:127, :]
                    )
                    small.dma_start(
                        out_dram[b : b + 1, PIECE1 + PIECE2 : OUT_COLS],
                        ev[127:128, 0:PIECE3],
                    )

    nc.compile()
    return nc


def _get_program():
    if "nc" not in _CACHED:
        _CACHED["nc"] = _build_program()
    return _CACHED["nc"]


def kernel(inputs: np.ndarray, _trace: bool = False, _tmpdir: str | None = None):
    from concourse.bass_utils import run_bass_kernel_spmd

    nc = _get_program()
    inputs = np.ascontiguousarray(np.asarray(inputs, dtype=np.int32))
    assert inputs.shape == (B_FULL, S), inputs.shape
    in_maps = [
        {"inputs": inputs[k * P : (k + 1) * P]} for k in range(N_CORES)
    ]
    res = run_bass_kernel_spmd(
        nc,
        in_maps,
        core_ids=list(range(N_CORES)),
        trace=_trace,
        tmpdir=_tmpdir,
    )
    out = np.concatenate([r["out"] for r in res.results], axis=0)
    if _trace:
        _CACHED["last_results"] = res
    return out
